# revision 17
# baseline (speedup 1.0000x reference)
"""GQA causal-attention prefill kernel for 8 TRN2 NeuronCores.

Sharding: tensor-parallel over heads. Core c owns q-heads {2c, 2c+1} and
kv-head c (whole GQA group). Each core computes its heads' attention and a
partial output projection; the host sums the 8 partials (no collectives).

Layout: activations kept transposed [feature, token] on-chip so every matmul
uses natural operand layouts. Matmuls run in bf16 (fast-weight-load path,
PSUM accumulation in f32). Softmax skips max-subtraction: scores are bounded
by ||q||*||k||/sqrt(D) <= sqrt(D)*max_gamma^2 ~ 12, safe in f32 exp.
rotate_half is a PE matmul against a signed permutation matrix.
"""
import os
import sys

for _p in ("/opt/trn_rl_repo", "/root/.axon_site/_ro/trn_rl_repo"):
    if os.path.isdir(_p) and _p not in sys.path:
        sys.path.append(_p)

import numpy as np
import ml_dtypes
import concourse.bacc as bacc
import concourse.bass_isa as bass_isa
import concourse.mybir as mybir
import concourse.tile as tile
from concourse.bass_utils import run_bass_kernel_spmd

B, S, DIM = 2, 2048, 2048
H, KVH, D = 16, 8, 128
EPS = 1e-6
NCORES = 8
HL = H // NCORES            # q heads per core
SQC = 512                   # sequence chunk (matmul moving dim)
NJ = S // SQC               # chunks per batch
KT = DIM // 128             # contraction tiles for the projections
BS = B * S
SCALE = 1.0 / float(np.sqrt(D))

F32 = mybir.dt.float32
BF16 = mybir.dt.bfloat16
AF = mybir.ActivationFunctionType


def build(debug_dumps=False):
    nc = bacc.Bacc("TRN2", target_bir_lowering=False, debug=False,
                   num_devices=NCORES)
    xt = nc.dram_tensor("xt", [DIM, BS], BF16, kind="ExternalInput").ap()
    wq = nc.dram_tensor("wq", [DIM, HL * D], BF16, kind="ExternalInput").ap()
    wk = nc.dram_tensor("wk", [DIM, D], BF16, kind="ExternalInput").ap()
    wv = nc.dram_tensor("wv", [DIM, D], BF16, kind="ExternalInput").ap()
    wo = nc.dram_tensor("wo", [HL * D, DIM], BF16, kind="ExternalInput").ap()
    gq = nc.dram_tensor("gq", [D, 1], F32, kind="ExternalInput").ap()
    gk = nc.dram_tensor("gk", [D, 1], F32, kind="ExternalInput").ap()
    cost = nc.dram_tensor("cost", [128, S], BF16, kind="ExternalInput").ap()
    sint = nc.dram_tensor("sint", [128, S], BF16, kind="ExternalInput").ap()
    msk = nc.dram_tensor("msk", [128, 128], BF16, kind="ExternalInput").ap()
    idn = nc.dram_tensor("idn", [128, 128], BF16, kind="ExternalInput").ap()
    rmt = nc.dram_tensor("rmt", [128, 128], BF16, kind="ExternalInput").ap()
    out = nc.dram_tensor("out", [DIM, BS], BF16, kind="ExternalOutput").ap()
    dbg = {}
    if debug_dumps:
        for nm in ("qt0d", "ktd", "vbd", "ao0d"):
            dbg[nm] = nc.dram_tensor(nm, [128, S], BF16,
                                     kind="ExternalOutput").ap()

    with tile.TileContext(nc) as tc:
        with tc.tile_pool(name="const", bufs=1) as cp, \
             tc.tile_pool(name="xp", bufs=24) as xp, \
             tc.tile_pool(name="persist", bufs=2) as pp, \
             tc.tile_pool(name="wrk", bufs=4) as wrk, \
             tc.tile_pool(name="ep", bufs=8) as epool, \
             tc.tile_pool(name="oop", bufs=4) as oop, \
             tc.tile_pool(name="ps_big", bufs=5, space="PSUM") as psb, \
             tc.tile_pool(name="ps_sm", bufs=1, space="PSUM") as pssm, \
             tc.tile_pool(name="ps_av", bufs=2, space="PSUM") as psa:

            # ---- constants / weights in SBUF ----
            wq_sb = cp.tile([128, KT * HL * D], BF16)
            wk_sb = cp.tile([128, KT * D], BF16)
            wv_sb = cp.tile([128, KT * D], BF16)
            for kt in range(KT):
                nc.gpsimd.dma_start(out=wq_sb[:, kt * HL * D:(kt + 1) * HL * D],
                                  in_=wq[kt * 128:(kt + 1) * 128, :])
                nc.gpsimd.dma_start(out=wk_sb[:, kt * D:(kt + 1) * D],
                                  in_=wk[kt * 128:(kt + 1) * 128, :])
                nc.gpsimd.dma_start(out=wv_sb[:, kt * D:(kt + 1) * D],
                                  in_=wv[kt * 128:(kt + 1) * 128, :])
            wo_sb = [cp.tile([128, DIM], BF16, name=f"wo{h}") for h in range(HL)]
            for h in range(HL):
                nc.gpsimd.dma_start(out=wo_sb[h][:], in_=wo[h * 128:(h + 1) * 128, :])
            cos_sb = cp.tile([128, S], BF16)
            sin_sb = cp.tile([128, S], BF16)
            nc.gpsimd.dma_start(out=cos_sb[:], in_=cost)
            nc.gpsimd.dma_start(out=sin_sb[:], in_=sint)
            msk_sb = cp.tile([128, 128], BF16)
            nc.gpsimd.dma_start(out=msk_sb[:], in_=msk)
            idn_sb = cp.tile([128, 128], BF16)
            nc.gpsimd.dma_start(out=idn_sb[:], in_=idn)
            rmt_sb = cp.tile([128, 128], BF16)
            nc.gpsimd.dma_start(out=rmt_sb[:], in_=rmt)
            gq_sb = cp.tile([D, 1], F32)
            gk_sb = cp.tile([D, 1], F32)
            nc.gpsimd.dma_start(out=gq_sb[:], in_=gq)
            nc.gpsimd.dma_start(out=gk_sb[:], in_=gk)
            eps_sb = cp.tile([128, 1], F32)
            nc.gpsimd.memset(eps_sb[:], EPS)

            for b in range(B):
                base = b * S
                # per-batch persistent buffers (double-buffered across b)
                qt_buf = [pp.tile([128, S], BF16, tag=f"qt{h}", name=f"qt{h}")
                          for h in range(HL)]
                kt_buf = pp.tile([128, S], BF16, tag="ktb")
                v_buf = pp.tile([128, S], BF16, tag="vb")
                ao_buf = [pp.tile([128, S], BF16, tag=f"ao{h}", name=f"ao{h}")
                          for h in range(HL)]
                # ======== phase P: projections + norm + rope ========
                for sc in range(NJ):
                    col = sc * SQC
                    xk = []
                    for kt in range(KT):
                        t = xp.tile([128, SQC], BF16, tag="x", name=f"x{kt}")
                        nc.sync.dma_start(
                            out=t[:],
                            in_=xt[kt * 128:(kt + 1) * 128,
                                   base + col:base + col + SQC])
                        xk.append(t)
                    qp = [psb.tile([128, SQC], F32, tag="big", name=f"qp{h}")
                          for h in range(HL)]
                    kp = psb.tile([128, SQC], F32, tag="big")
                    vp = psb.tile([128, SQC], F32, tag="big")
                    for kt in range(KT):
                        st, sp = kt == 0, kt == KT - 1
                        for h in range(HL):
                            nc.tensor.matmul(
                                qp[h][:],
                                wq_sb[:, kt * HL * D + h * D:kt * HL * D + (h + 1) * D],
                                xk[kt][:], start=st, stop=sp)
                        nc.tensor.matmul(kp[:], wk_sb[:, kt * D:(kt + 1) * D],
                                         xk[kt][:], start=st, stop=sp)
                        nc.tensor.matmul(vp[:], wv_sb[:, kt * D:(kt + 1) * D],
                                         xk[kt][:], start=st, stop=sp)

                    # -- rmsnorm + gamma + rope for q heads and k --
                    for ps, g_col, dest in (
                            [(qp[h], gq_sb, qt_buf[h]) for h in range(HL)]
                            + [(kp, gk_sb, kt_buf)]):
                        sqr = wrk.tile([128, SQC], BF16, tag="sqr")
                        nc.scalar.activation(sqr[:], ps[:], AF.Square)
                        tsb = wrk.tile([128, SQC], BF16, tag="tsb")
                        nc.scalar.activation(tsb[:], ps[:], AF.Copy,
                                             scale=g_col[:])
                        ssbc = wrk.tile([128, SQC], F32, tag="ssbc")
                        nc.gpsimd.partition_all_reduce(ssbc[:], sqr[:], 128,
                                                       bass_isa.ReduceOp.add)
                        sdn = wrk.tile([128, SQC], F32, tag="ssbc")
                        nc.scalar.activation(sdn[:], ssbc[:], AF.Sqrt,
                                             scale=1.0 / D, bias=eps_sb[:])
                        bcs = wrk.tile([128, SQC], F32, tag="bcs")
                        nc.vector.reciprocal_approx_fast(bcs[:], sdn[:])
                        rot = pssm.tile([128, SQC], F32, tag="sm")
                        nc.tensor.matmul(rot[:], rmt_sb[:], tsb[:],
                                         start=True, stop=True)
                        rots = wrk.tile([128, SQC], BF16, tag="rots")
                        nc.vector.tensor_copy(rots[:], rot[:])
                        c_sl = cos_sb[:, col:col + SQC]
                        s_sl = sin_sb[:, col:col + SQC]
                        m1 = wrk.tile([128, SQC], BF16, tag="m1")
                        m2 = wrk.tile([128, SQC], BF16, tag="m2")
                        nc.vector.tensor_mul(m1[:], tsb[:], c_sl)
                        nc.vector.tensor_mul(m2[:], rots[:], s_sl)
                        u = wrk.tile([128, SQC], BF16, tag="m1")
                        nc.vector.tensor_add(u[:], m1[:], m2[:])
                        nc.vector.tensor_mul(dest[:, col:col + SQC],
                                             u[:], bcs[:])

                    # -- V: copy + transpose to natural [s, d] layout --
                    vt = wrk.tile([128, SQC], BF16, tag="tsb")
                    nc.scalar.activation(vt[:], vp[:], AF.Copy)
                    for cq in range(4):
                        vq = pssm.tile([128, 128], BF16, tag="sm")
                        nc.tensor.transpose(vq[:], vt[:, cq * 128:(cq + 1) * 128],
                                            idn_sb[:])
                        ti = sc * 4 + cq
                        nc.vector.tensor_copy(
                            v_buf[:, ti * 128:(ti + 1) * 128], vq[:])

                # ======== phase A: attention ========
                for h in range(HL):
                    for j in range(NJ):
                        nsk = 4 * j + 4
                        avp = psa.tile([128, SQC], F32, tag="av")
                        esum = wrk.tile([128, SQC], F32, tag="esum")
                        for i in range(nsk):
                            lo = max(0, (i - 4 * j) * 128)
                            scp = psb.tile([128, SQC], F32, tag="big")
                            nc.tensor.matmul(
                                scp[:, lo:], kt_buf[:, i * 128:(i + 1) * 128],
                                qt_buf[h][:, j * SQC + lo:(j + 1) * SQC],
                                start=True, stop=True)
                            e = epool.tile([128, SQC], BF16, tag="e")
                            nc.scalar.activation(e[:, lo:], scp[:, lo:],
                                                 AF.Exp, scale=SCALE)
                            if i >= 4 * j:
                                nc.vector.tensor_mul(
                                    e[:, lo:lo + 128], e[:, lo:lo + 128],
                                    msk_sb[:, 0:128])
                            st, sp = i == 0, i == nsk - 1
                            nc.tensor.matmul(avp[:, lo:],
                                             v_buf[:, i * 128:(i + 1) * 128],
                                             e[:, lo:], start=st, stop=sp)
                            if i == 0:
                                nc.vector.tensor_copy(esum[:], e[:])
                            else:
                                nc.vector.tensor_add(esum[:, lo:],
                                                     esum[:, lo:], e[:, lo:])
                        dbc = wrk.tile([128, SQC], F32, tag="esum")
                        nc.gpsimd.partition_all_reduce(dbc[:], esum[:], 128,
                                                       bass_isa.ReduceOp.add)
                        rec = wrk.tile([128, SQC], F32, tag="bcs")
                        nc.vector.reciprocal_approx_fast(rec[:], dbc[:])
                        avs = wrk.tile([128, SQC], BF16, tag="sqr")
                        nc.scalar.activation(avs[:], avp[:], AF.Copy)
                        nc.vector.tensor_mul(
                            ao_buf[h][:, j * SQC:(j + 1) * SQC],
                            avs[:], rec[:])

                if debug_dumps and b == 0:
                    nc.sync.dma_start(out=dbg["qt0d"], in_=qt_buf[0][:])
                    nc.sync.dma_start(out=dbg["ktd"], in_=kt_buf[:])
                    nc.sync.dma_start(out=dbg["vbd"], in_=v_buf[:])
                    nc.sync.dma_start(out=dbg["ao0d"], in_=ao_buf[0][:])

                # ======== phase O: output projection (partial) ========
                for j in range(NJ):
                    for dt in range(KT):
                        op = psb.tile([128, SQC], F32, tag="big")
                        for h in range(HL):
                            nc.tensor.matmul(
                                op[:], wo_sb[h][:, dt * 128:(dt + 1) * 128],
                                ao_buf[h][:, j * SQC:(j + 1) * SQC],
                                start=(h == 0), stop=(h == HL - 1))
                        oo = oop.tile([128, SQC], BF16, tag="oo")
                        if dt % 2 == 0:
                            nc.scalar.activation(oo[:], op[:], AF.Copy)
                        else:
                            nc.vector.tensor_copy(oo[:], op[:])
                        nc.sync.dma_start(
                            out=out[dt * 128:(dt + 1) * 128,
                                    base + j * SQC:base + (j + 1) * SQC],
                            in_=oo[:])
    nc.compile()
    return nc


_NC_CACHE = None


def _get_nc():
    global _NC_CACHE
    if _NC_CACHE is None:
        _NC_CACHE = build()
    return _NC_CACHE


def _bf(a):
    return np.ascontiguousarray(a.astype(ml_dtypes.bfloat16))


def kernel(x, wq, wk, wv, wo, q_gamma, k_gamma, cos_cache, sin_cache):
    x = np.asarray(x, dtype=np.float32)
    wq = np.asarray(wq, dtype=np.float32)
    wk = np.asarray(wk, dtype=np.float32)
    wv = np.asarray(wv, dtype=np.float32)
    wo = np.asarray(wo, dtype=np.float32)
    q_gamma = np.asarray(q_gamma, dtype=np.float32)
    k_gamma = np.asarray(k_gamma, dtype=np.float32)
    cos_cache = np.asarray(cos_cache, dtype=np.float32)
    sin_cache = np.asarray(sin_cache, dtype=np.float32)

    xt = _bf(x.reshape(BS, DIM).T)
    cos_t = cos_cache[:S].T
    sin_t = sin_cache[:S].T
    cost = _bf(np.concatenate([cos_t, cos_t], axis=0))
    sint = _bf(np.concatenate([sin_t, sin_t], axis=0))
    gq = np.ascontiguousarray(q_gamma[:, None])
    gk = np.ascontiguousarray(k_gamma[:, None])
    p = np.arange(128)[:, None]
    c = np.arange(128)[None, :]
    msk = _bf((p <= c).astype(np.float32))
    idn = _bf(np.eye(128, dtype=np.float32))
    pmat = np.zeros((128, 128), np.float32)
    pmat[np.arange(64), np.arange(64) + 64] = -1.0
    pmat[np.arange(64) + 64, np.arange(64)] = 1.0
    rmt = _bf(pmat.T)

    in_maps = []
    for cid in range(NCORES):
        in_maps.append({
            "xt": xt,
            "wq": _bf(wq[:, cid * HL * D:(cid + 1) * HL * D]),
            "wk": _bf(wk[:, cid * D:(cid + 1) * D]),
            "wv": _bf(wv[:, cid * D:(cid + 1) * D]),
            "wo": _bf(wo[cid * HL * D:(cid + 1) * HL * D, :]),
            "gq": gq, "gk": gk, "cost": cost, "sint": sint,
            "msk": msk, "idn": idn, "rmt": rmt,
        })

    nc = _get_nc()
    trace = os.environ.get("KERNEL_TRACE") == "1"
    r = run_bass_kernel_spmd(nc, in_maps, core_ids=list(range(NCORES)),
                             trace=trace)
    if trace:
        kernel.last_exec_time_ns = r.exec_time_ns
        kernel.last_results = r
    acc = np.zeros((DIM, BS), np.float32)
    for cid in range(NCORES):
        acc += r.results[cid]["out"].astype(np.float32)
    return np.ascontiguousarray(
        acc.T.reshape(B, S, DIM).astype(np.float32))
